# revision 31
# baseline (speedup 1.0000x reference)
"""Trainium2 Bass kernel for nn_Aspect_Attention_op2 (B=16, L=2048, D=768).

reference semantics:
    y = tanh(x2 @ att_W)                        # [B, L, D]
    wlog = einsum('d,bld->bl', att_v, y)        # [B, L]
    w = softmax(wlog, axis=0)                   # softmax over BATCH
    w_tiled[b,i,j] = w[b, (i*D+j) % L]          # tile-then-reshape
    out = x2 * w_tiled
    score = x @ out^T ; attn = softmax(score, -1) ; ctx = attn @ out

Distribution: batch-parallel, 2 batches/core on 8 cores. The batch softmax
needs one 8KB AllReduce(add) of sum_b exp(wlog) (max-subtraction is skipped:
logits are ~N(0, 0.08), scores |.| < ~35 -> fp32 exp is exact enough).

Layout tricks:
  * w_tiled multiply in natural layout == view x2[b] flat as [768, 2048] and
    scale columns by w[b] (same DRAM bytes, different AP).
  * in TRANSPOSED layout the multiplier has period 8 along k:
    outT[j, k] = x2T[j, k] * w[(768*(k%8) + j) % 2048], so outT is computed
    in-place on a resident x2T tile with 48 strided tensor_scalar_muls
    against a [128, 6, 8] multiplier tile M (built via one DMA from a
    4x-duplicated w row in DRAM + 6 PE transposes).
  * attention is computed as scoreT[k, q] = outT.T @ xT so that
    exp(scoreT) is directly the lhsT of the PV matmul (no transposes of attn),
    and the softmax denominator comes from a ones-column appended to V.
  * xT/x2T come from bf16 DMA-xbar transposes of a bf16 scratch copy.
"""

import sys

try:
    import concourse  # noqa: F401
except ImportError:
    sys.path.insert(0, "/opt/trn_rl_repo")

import numpy as np

import concourse.bass as bass
import concourse.bacc as bacc
import concourse.mybir as mybir
import concourse.tile as tile
from concourse.bass_utils import run_bass_kernel_spmd

B, L, D = 16, 2048, 768
NCORES = 8
NB = B // NCORES          # batches per core = 2
P = 128
DT = D // P               # 6 d-tiles
KT = L // P               # 16 k-tiles
QC = 512                  # q-chunk (psum free dim)
NQC = L // QC             # 4 q-chunks
FP32 = mybir.dt.float32
BF16 = mybir.dt.bfloat16
AF = mybir.ActivationFunctionType


def ts(i, n):
    return bass.ts(i, n)


def build_nc():
    nc = bacc.Bacc("TRN2", target_bir_lowering=False, debug=False,
                   num_devices=NCORES)

    x_ext = nc.dram_tensor("x", [NB, L, D], FP32, kind="ExternalInput")
    x2_ext = nc.dram_tensor("x2", [NB, L, D], FP32, kind="ExternalInput")
    v_ext = nc.dram_tensor("att_v", [D], FP32, kind="ExternalInput")
    w_ext = nc.dram_tensor("att_W", [D, D], FP32, kind="ExternalInput")
    out_ext = nc.dram_tensor("out", [NB, L, D], FP32, kind="ExternalOutput")

    ar_out = nc.dram_tensor("ar_out", [1, L], FP32, addr_space="Shared")

    with tile.TileContext(nc) as tc:
        _body(nc, tc, x_ext, x2_ext, v_ext, w_ext, out_ext, ar_out)
    nc.compile()
    return nc


def _cast_pass(nc, cast_in, cast_out, src, dst, nchunks=16):
    """f32 [L, D] -> bf16 scratch; two parallel chains (DVE + ACT)."""
    rows = L // nchunks
    g = rows // P
    for c in range(nchunks):
        eng = "v" if c % 2 == 0 else "s"
        sl = src[ts(c, rows), :].rearrange("(g p) d -> p g d", p=P)
        dl = dst[ts(c, rows), :].rearrange("(g p) d -> p g d", p=P)
        cf = cast_in.tile([P, g, D], FP32, tag="cast", name="cf")
        nc.sync.dma_start(out=cf[:], in_=sl)
        cb = cast_out.tile([P, g, D], BF16, tag="castb", name="cb")
        if eng == "v":
            nc.vector.tensor_copy(cb[:], cf[:])
        else:
            nc.scalar.copy(cb[:], cf[:])
        nc.sync.dma_start(out=dl, in_=cb[:])


def _body(nc, tc, x_ext, x2_ext, v_ext, w_ext, out_ext, ar_out):
    from contextlib import ExitStack

    with ExitStack() as st:
        pool = lambda name, bufs, **kw: st.enter_context(
            tc.tile_pool(name=name, bufs=bufs, **kw))

        const = pool("const", 1)
        rows_p = pool("rows_p", 1)
        rows_t = pool("rows_t", 1)
        cast_in = pool("cast_in", 3)
        cast_out = pool("cast_out", 2)
        x2s_p = pool("x2s_p", 2)     # streamed x2T chunks for the y phase
        x2t_p = pool("x2t_p", 1)     # full x2T per batch; becomes outT in place
        xt_p = pool("xt_p", 1)       # full xT per batch
        yt_p = pool("yt_p", 1)
        wb_p = pool("wb_p", 1)
        flat_p = pool("flat_p", 1)
        oflat_p = pool("oflat_p", 1)
        oa_p = pool("oa_p", 1)
        expT_p = pool("expT_p", 18)
        ctx_p = pool("ctx_p", 1)
        rec_p = pool("rec_p", 3)
        m_p = pool("m_p", 1)

        psum_a = pool("psum_a", 3, space="PSUM")
        psum_b = pool("psum_b", 2, space="PSUM")
        psum_c = pool("psum_c", 2, space="PSUM")
        psum_w = pool("psum_w", 1, space="PSUM")

        dram = pool("dram", 1, space="DRAM")

        # ---- DRAM scratch ----
        x2bf = [dram.tile([L, D], BF16, name=f"x2bf{b}") for b in range(NB)]
        xbf = [dram.tile([L, D], BF16, name=f"xbf{b}") for b in range(NB)]
        outbf = [dram.tile([L, D], BF16, name=f"outbf{b}") for b in range(NB)]
        ar_in = dram.tile([1, L], FP32, name="ar_in")
        recd = dram.tile([1, L], FP32, name="recd")

        # ---- constants ----
        W_sb = const.tile([P, DT, D], BF16)   # W[d, e] bf16
        for dt in range(DT):
            wf = cast_in.tile([P, D], FP32, tag="cast", name="wf")
            nc.sync.dma_start(out=wf[:], in_=w_ext[ts(dt, P), :])
            nc.vector.tensor_copy(W_sb[:, dt, :], wf[:])
        v_sb = const.tile([P, DT], BF16)
        vf = cast_in.tile([P, DT], FP32, tag="cast", name="vf")
        nc.sync.dma_start(
            out=vf[:], in_=v_ext.ap().rearrange("(a p) -> p a", p=P))
        nc.vector.tensor_copy(v_sb[:], vf[:])
        ones_sb = const.tile([1, P], FP32)
        nc.vector.memset(ones_sb[:], 1.0)
        one1 = const.tile([1, 1], FP32)
        nc.vector.memset(one1[:], 1.0)


        exp_wlog = [rows_p.tile([1, L], BF16, name=f"ewl{b}") for b in range(NB)]
        recip = rows_p.tile([1, L], FP32, name="recip")

        # ---- per kc: cast x2+x tiles -> transpose chunks -> y matmuls ----
        x2T0 = x2t_p.tile([P, DT, L], BF16, name="x2T0")
        xT0 = xt_p.tile([P, DT, L], BF16, name="xT0")
        for b in range(NB):
            for kc in range(NQC):
                for c in range(4 * kc, 4 * kc + 4):
                    for src_e, dst_e in ((x2_ext, x2bf), (x_ext, xbf)):
                        eng = "v" if c % 2 == 0 else "s"
                        cf = cast_in.tile([P, D], FP32, tag="cast",
                                          name="cf")
                        nc.sync.dma_start(out=cf[:],
                                          in_=src_e[b][ts(c, P), :])
                        cb = cast_out.tile([P, D], BF16, tag="castb",
                                           name="cb")
                        if eng == "v":
                            nc.vector.tensor_copy(cb[:], cf[:])
                        else:
                            nc.scalar.copy(cb[:], cf[:])
                        nc.sync.dma_start(out=dst_e[b][ts(c, P), :],
                                          in_=cb[:])
                x2s = x2s_p.tile([P, DT, QC], BF16, name="x2s")
                for dt in range(DT):
                    nc.sync.dma_start_transpose(
                        x2s[:, dt, :], x2bf[b][ts(kc, QC), ts(dt, P)])
                if b == 0:
                    for dt in range(DT):
                        nc.sync.dma_start_transpose(
                            x2T0[:, dt, ts(kc, QC)],
                            x2bf[0][ts(kc, QC), ts(dt, P)])
                    for dt in range(DT):
                        nc.sync.dma_start_transpose(
                            xT0[:, dt, ts(kc, QC)],
                            xbf[0][ts(kc, QC), ts(dt, P)])
                pw = psum_w.tile([1, QC], FP32, tag="psw", name="pw")
                for ep in range(DT // 2):
                    yt = yt_p.tile([P, 2, QC], BF16, name="yt")
                    for e2 in range(2):
                        et = 2 * ep + e2
                        ps = psum_a.tile([P, QC], FP32, tag="psa",
                                         name="ps_y")
                        for dt in range(DT):
                            nc.tensor.matmul(
                                ps[:], W_sb[:, dt, ts(et, P)], x2s[:, dt, :],
                                start=(dt == 0), stop=(dt == DT - 1))
                        nc.scalar.activation(yt[:, e2, :], ps[:], AF.Tanh)
                    for e2 in range(2):
                        et = 2 * ep + e2
                        nc.tensor.matmul(
                            pw[:], v_sb[:, et:et + 1], yt[:, e2, :],
                            start=(et == 0), stop=(et == DT - 1))
                nc.scalar.activation(
                    exp_wlog[b][:, ts(kc, QC)], pw[:], AF.Exp)

        # ---- AllReduce of sum_b exp(wlog) ----
        partial = rows_t.tile([1, L], FP32, tag="row", name="partial")
        nc.vector.tensor_add(partial[:], exp_wlog[0][:], exp_wlog[1][:])
        nc.sync.dma_start(out=ar_in[:], in_=partial[:])
        nc.gpsimd.collective_compute(
            "AllReduce", mybir.AluOpType.add,
            replica_groups=[list(range(NCORES))],
            ins=[ar_in[:].opt()], outs=[ar_out.ap().opt()])

        # denom load on the scalar queue so its wait blocks nothing else;
        # reciprocal in [128, 16] layout (single-lane recip is ~15us),
        # scattered back to a row via DRAM
        denom_pm = m_p.tile([P, 16], FP32, tag="dpm", name="denom_pm")
        nc.scalar.dma_start(
            out=denom_pm[:],
            in_=ar_out.ap()[0, :].rearrange("(p i) -> p i", i=16))
        recip_pm = m_p.tile([P, 16], FP32, tag="rpm", name="recip_pm")
        nc.vector.reciprocal(recip_pm[:], denom_pm[:])
        nc.scalar.dma_start(
            out=recd[0, :].rearrange("(p i) -> p i", i=16), in_=recip_pm[:])
        nc.scalar.dma_start(out=recip[:], in_=recd[:])

        x2T = [None] * NB
        xT = [None] * NB
        x2T[0] = x2T0
        xT[0] = xT0

        # ---- per batch: weights, outT (in place on x2T), out, attention ----
        for b in range(NB):
            w_row = rows_t.tile([1, L], FP32, tag="row", name=f"w_row{b}")
            nc.vector.tensor_mul(w_row[:], exp_wlog[b][:], recip[:])

            # M[p, jt, r] = w_row[(128jt + 768r) % 2048 + p] via 48 K=1
            # matmuls (the offsets are 128-aligned and never wrap)
            M_sb = m_p.tile([P, DT, 8], FP32, tag="m", name="M_sb")
            for jt in range(DT):
                pm = psum_w.tile([P, 8], FP32, tag="psw", name="pm")
                for r in range(8):
                    o = (128 * jt + 768 * r) % L
                    nc.tensor.matmul(pm[:, r:r + 1], w_row[:, o:o + P],
                                     one1[:], start=True, stop=True)
                nc.vector.tensor_copy(M_sb[:, jt, :], pm[:])
            # outT in place: x2T[:, jt, r::8] *= M[:, jt, r]
            # split across DVE (tensor_scalar) and ACT (Copy with scale=)
            for jt in range(DT):
                sl = x2T[b][:, jt, :].rearrange("p (k e) -> p k e", e=8)
                for r in range(8):
                    if r % 2 == 0:
                        nc.vector.tensor_scalar_mul(
                            sl[:, :, r], sl[:, :, r], M_sb[:, jt, r:r + 1])
                    else:
                        nc.scalar.activation(
                            sl[:, :, r], sl[:, :, r], AF.Copy,
                            scale=M_sb[:, jt, r:r + 1])
            outT = x2T[b]

            # broadcast w to 128 partitions (bf16) for the flat multiply
            wb = wb_p.tile([P, L], BF16, name="wb")
            for c in range(NQC):
                psb = psum_a.tile([P, QC], FP32, tag="psa", name="psb")
                nc.tensor.matmul(psb[:], ones_sb[:], w_row[:, ts(c, QC)],
                                 start=True, stop=True)
                nc.vector.tensor_copy(wb[:, ts(c, QC)], psb[:])
            x2fl = x2bf[b][:].rearrange("l d -> (l d)").rearrange(
                "(r c) -> r c", c=L)
            ofl = outbf[b][:].rearrange("l d -> (l d)").rearrange(
                "(r c) -> r c", c=L)
            for j in range(DT):
                for h in range(4):
                    hs = ts(h, L // 4)
                    xf = flat_p.tile([P, L // 4], BF16, name="xf")
                    nc.sync.dma_start(out=xf[:], in_=x2fl[ts(j, P), hs])
                    of = oflat_p.tile([P, L // 4], BF16, name="of")
                    nc.vector.tensor_mul(of[:], xf[:], wb[:, hs])
                    nc.sync.dma_start(out=ofl[ts(j, P), hs], in_=of[:])

            # V with ones column: oa[128, kt, 769]
            oa = oa_p.tile([P, KT, D + 1], BF16, name="oa")
            for g in range(4):   # 4 k-tiles per DMA
                src = outbf[b][ts(g, 4 * P), :].rearrange(
                    "(t p) d -> p t d", p=P)
                nc.sync.dma_start(out=oa[:, 4 * g:4 * g + 4, 0:D], in_=src)
            nc.vector.memset(oa[:, :, D:D + 1], 1.0)

            for qc in range(NQC):
                # QK: per-kt expT tiles (ring of 16) so the next qc's QK can
                # start overwriting slots as PV consumes them kt-major
                expT = []
                for kt in range(KT):
                    ps = psum_a.tile([P, QC], FP32, tag="psa", name="ps_qk")
                    for dt in range(DT):
                        nc.tensor.matmul(
                            ps[:], outT[:, dt, ts(kt, P)],
                            xT[b][:, dt, ts(qc, QC)],
                            start=(dt == 0), stop=(dt == DT - 1))
                    et = expT_p.tile([P, QC], BF16, tag="expT", name="et")
                    nc.scalar.activation(et[:], ps[:], AF.Exp)
                    expT.append(et)
                # PV: kt-major over pairs of q-tiles (4 psum tiles live)
                for qh in range(QC // P // 2):
                    pcs = []
                    for qt in (2 * qh, 2 * qh + 1):
                        pc1 = psum_b.tile([P, 512], FP32, tag="psb",
                                          name="pc1")
                        pc2 = psum_c.tile([P, 257], FP32, tag="psc",
                                          name="pc2")
                        pcs.append((qt, pc1, pc2))
                    for kt in range(KT):
                        for qt, pc1, pc2 in pcs:
                            lh = expT[kt][:, ts(qt, P)]
                            nc.tensor.matmul(
                                pc1[:], lh, oa[:, kt, 0:512],
                                start=(kt == 0), stop=(kt == KT - 1))
                            nc.tensor.matmul(
                                pc2[:], lh, oa[:, kt, 512:D + 1],
                                start=(kt == 0), stop=(kt == KT - 1))
                    for qt, pc1, pc2 in pcs:
                        rec = rec_p.tile([P, 1], FP32, name="rec")
                        nc.vector.reciprocal(rec[:], pc2[:, 256:257])
                        cc = ctx_p.tile([P, D], FP32, tag="cc", name="cc")
                        nc.vector.tensor_scalar_mul(
                            cc[:, 0:512], pc1[:], rec[:])
                        nc.vector.tensor_scalar_mul(
                            cc[:, 512:D], pc2[:, 0:256], rec[:])
                        q0 = qc * QC + qt * P
                        nc.sync.dma_start(
                            out=out_ext[b, q0:q0 + P, :], in_=cc[:])
                # prefetch next batch x2T/xT at the tail of b0's attention
                if b == 0 and qc == NQC - 1:
                    x2T[1] = x2t_p.tile([P, DT, L], BF16, name="x2T1")
                    xT[1] = xt_p.tile([P, DT, L], BF16, name="xT1")
                    for dt in range(DT):
                        nc.sync.dma_start_transpose(
                            x2T[1][:, dt, :], x2bf[1][:, ts(dt, P)])
                    for dt in range(DT):
                        nc.sync.dma_start_transpose(
                            xT[1][:, dt, :], xbf[1][:, ts(dt, P)])


_NC_CACHE = None


def kernel(x, x2, att_v, att_W):
    global _NC_CACHE
    if _NC_CACHE is None:
        _NC_CACHE = build_nc()
    nc = _NC_CACHE

    x = np.ascontiguousarray(x, dtype=np.float32)
    x2 = np.ascontiguousarray(x2, dtype=np.float32)
    att_v = np.ascontiguousarray(att_v, dtype=np.float32)
    att_W = np.ascontiguousarray(att_W, dtype=np.float32)

    in_maps = []
    for i in range(NCORES):
        sl = slice(i * NB, (i + 1) * NB)
        in_maps.append({
            "x": x[sl], "x2": x2[sl], "att_v": att_v, "att_W": att_W,
        })
    res = run_bass_kernel_spmd(nc, in_maps, core_ids=list(range(NCORES)))
    outs = [res.results[i]["out"] for i in range(NCORES)]
    return np.concatenate(outs, axis=0).astype(np.float32)


if __name__ == "__main__":
    xs = np.random.randn(B, L, D).astype(np.float32)
    x2s = np.random.randn(B, L, D).astype(np.float32)
    vs = (np.random.randn(D) * 0.01).astype(np.float32)
    Ws = (np.random.randn(D, D) * 0.01).astype(np.float32)
    o = kernel(x=xs, x2=x2s, att_v=vs, att_W=Ws)
    print(o.shape, o.dtype)


# revision 36
# speedup vs baseline: 1.1007x; 1.1007x over previous
"""Trainium2 Bass kernel for nn_Aspect_Attention_op2 (B=16, L=2048, D=768).

reference semantics:
    y = tanh(x2 @ att_W)                        # [B, L, D]
    wlog = einsum('d,bld->bl', att_v, y)        # [B, L]
    w = softmax(wlog, axis=0)                   # softmax over BATCH
    w_tiled[b,i,j] = w[b, (i*D+j) % L]          # tile-then-reshape
    out = x2 * w_tiled
    score = x @ out^T ; attn = softmax(score, -1) ; ctx = attn @ out

Distribution: batch-parallel, 2 batches/core on 8 cores. The batch softmax
needs one 8KB AllReduce(add) of sum_b exp(wlog) (max-subtraction is skipped:
logits are ~N(0, 0.08), scores |.| < ~35 -> fp32 exp is exact enough).

Layout tricks:
  * w_tiled multiply in natural layout == view x2[b] flat as [768, 2048] and
    scale columns by w[b] (same DRAM bytes, different AP).
  * in TRANSPOSED layout the multiplier has period 8 along k:
    outT[j, k] = x2T[j, k] * w[(768*(k%8) + j) % 2048]; outT is computed
    in-place on the resident x2T tile with 48 strided scalar-multiplies
    against a [128, 6, 8] multiplier tile M. M itself is built with 48 K=1
    rank-1 matmuls from 128-aligned slices of the w row (never wraps).
  * attention is computed as scoreT[k, q] = outT.T @ xT so that
    exp(scoreT) is directly the lhsT of the PV matmul (no transposes of attn),
    and the softmax denominator comes from a ones-column appended to V.
  * transposes go through the idle TensorEngine (is_transpose matmuls vs a
    one-time identity built from 128 K=1 matmuls) instead of DMA-xbar --
    DMA-transpose issue costs ~1.3us of sequencer time each and chokes the
    front end. Only batch-1's xT uses DMA transposes, prefetched at the tail
    of batch-0's attention when the sequencer is idle.
"""

import sys

try:
    import concourse  # noqa: F401
except ImportError:
    sys.path.insert(0, "/opt/trn_rl_repo")

import numpy as np

import concourse.bass as bass
import concourse.bacc as bacc
import concourse.mybir as mybir
import concourse.tile as tile
from concourse.bass_utils import run_bass_kernel_spmd

B, L, D = 16, 2048, 768
NCORES = 8
NB = B // NCORES          # batches per core = 2
P = 128
DT = D // P               # 6 d-tiles
KT = L // P               # 16 k-tiles
QC = 512                  # q-chunk (psum free dim)
NQC = L // QC             # 4 q-chunks
FP32 = mybir.dt.float32
BF16 = mybir.dt.bfloat16
AF = mybir.ActivationFunctionType


def ts(i, n):
    return bass.ts(i, n)


def build_nc():
    nc = bacc.Bacc("TRN2", target_bir_lowering=False, debug=False,
                   num_devices=NCORES)

    x_ext = nc.dram_tensor("x", [NB, L, D], FP32, kind="ExternalInput")
    x2_ext = nc.dram_tensor("x2", [NB, L, D], FP32, kind="ExternalInput")
    v_ext = nc.dram_tensor("att_v", [D], FP32, kind="ExternalInput")
    w_ext = nc.dram_tensor("att_W", [D, D], FP32, kind="ExternalInput")
    out_ext = nc.dram_tensor("out", [NB, L, D], FP32, kind="ExternalOutput")

    ar_out = nc.dram_tensor("ar_out", [1, L], FP32, addr_space="Shared")

    with tile.TileContext(nc) as tc:
        _body(nc, tc, x_ext, x2_ext, v_ext, w_ext, out_ext, ar_out)
    nc.compile()
    return nc


def _body(nc, tc, x_ext, x2_ext, v_ext, w_ext, out_ext, ar_out):
    from contextlib import ExitStack

    with ExitStack() as st:
        pool = lambda name, bufs, **kw: st.enter_context(
            tc.tile_pool(name=name, bufs=bufs, **kw))

        const = pool("const", 1)
        rows_p = pool("rows_p", 1)
        rows_t = pool("rows_t", 1)
        cast_in = pool("cast_in", 3)
        cast_out = pool("cast_out", 2)
        x2t_p = pool("x2t_p", 1)
        x2s_p = pool("x2s_p", 2)     # full x2T per batch; becomes outT in place
        xt_p = pool("xt_p", 1)       # full xT per batch
        yt_p = pool("yt_p", 1)
        wb_p = pool("wb_p", 1)
        flat_p = pool("flat_p", 1)
        oflat_p = pool("oflat_p", 1)
        oa_p = pool("oa_p", 1)
        expT_p = pool("expT_p", 16)
        ctx_p = pool("ctx_p", 1)
        rec_p = pool("rec_p", 2)
        m_p = pool("m_p", 1)

        psum_a = pool("psum_a", 3, space="PSUM")    # y + QK accumulators
        psum_bt = pool("psum_bt", 2, space="PSUM")  # PE transposes / PV pc1
        psum_cw = pool("psum_cw", 2, space="PSUM")  # wlog+M / PV pc2
        dram = pool("dram", 1, space="DRAM")

        # ---- DRAM scratch ----
        x2bf = [dram.tile([L, D], BF16, name=f"x2bf{b}") for b in range(NB)]
        xbf1 = dram.tile([L, D], BF16, name="xbf1")
        outbf = [dram.tile([L, D], BF16, name=f"outbf{b}") for b in range(NB)]
        ar_in = dram.tile([1, L], FP32, name="ar_in")
        recd = dram.tile([1, L], FP32, name="recd")

        # ---- constants ----
        W_sb = oa_p.tile([P, DT, D], BF16, tag="oa", name="W_sb")  # W[d, e]
        for dt in range(DT):
            wf = cast_in.tile([P, D], FP32, tag="cast", name="wf")
            nc.sync.dma_start(out=wf[:], in_=w_ext[ts(dt, P), :])
            nc.vector.tensor_copy(W_sb[:, dt, :], wf[:])
        v_sb = const.tile([P, DT], BF16)
        vf = cast_in.tile([P, DT], FP32, tag="cast", name="vf")
        nc.sync.dma_start(
            out=vf[:], in_=v_ext.ap().rearrange("(a p) -> p a", p=P))
        nc.vector.tensor_copy(v_sb[:], vf[:])
        ones_sb = const.tile([1, P], FP32)
        nc.vector.memset(ones_sb[:], 1.0)
        one1 = const.tile([1, 1], FP32)
        nc.vector.memset(one1[:], 1.0)

        # identity (bf16) for PE transposes: column c = delta(row - c),
        # built as 128 rank-1 K=1 matmuls from a shifted delta row
        drow = const.tile([1, 2 * P + 1], FP32)
        nc.vector.memset(drow[:], 0.0)
        nc.vector.memset(drow[:, P:P + 1], 1.0)
        id128 = const.tile([P, P], BF16)
        for h in range(2):
            pI = psum_bt.tile([P, P // 2], FP32, tag="bt", name="pI")
            for i in range(P // 2):
                c = h * (P // 2) + i
                nc.tensor.matmul(pI[:, i:i + 1], drow[:, P - c:2 * P - c],
                                 one1[:], start=True, stop=True)
            nc.vector.tensor_copy(id128[:, ts(h, P // 2)], pI[:])

        exp_wlog = [rows_p.tile([1, L], BF16, name=f"ewl{b}")
                    for b in range(NB)]
        recip = rows_p.tile([1, L], FP32, name="recip")

        x2T = [None] * NB
        x2T[0] = x2t_p.tile([P, DT, L], BF16, name="x2T0")
        xT = [None] * NB
        xT[0] = xt_p.tile([P, DT, L], BF16, name="xT0")

        # ---- phase 1 per (b, kc): cast tiles; PE-transpose x2T (and xT for
        # b0); y matmuls + wlog + exp ----
        for b in range(NB):
            for kc in range(NQC):
                for c in range(4 * kc, 4 * kc + 4):
                    # x2 tile: cast -> scratch + PE transpose into x2T
                    cf = cast_in.tile([P, D], FP32, tag="cast", name="cf")
                    nc.sync.dma_start(out=cf[:], in_=x2_ext[b][ts(c, P), :])
                    cb = cast_out.tile([P, D], BF16, tag="castb", name="cb")
                    if c % 2 == 0:
                        nc.vector.tensor_copy(cb[:], cf[:])
                    else:
                        nc.scalar.copy(cb[:], cf[:])
                    nc.sync.dma_start(out=x2bf[b][ts(c, P), :], in_=cb[:])
                    if b == 0:
                        for dt in range(DT):
                            pt = psum_bt.tile([P, P], BF16, tag="bt",
                                              name="pt")
                            nc.tensor.matmul(pt[:], cb[:, ts(dt, P)],
                                             id128[:], is_transpose=True,
                                             start=True, stop=True)
                            dst = x2T[0][:, dt, ts(c, P)]
                            if dt % 2 == 0:
                                nc.vector.tensor_copy(dst, pt[:])
                            else:
                                nc.scalar.copy(dst, pt[:])
                    # x tile: cast; b0 -> PE transpose into xT; b1 -> scratch
                    cfx = cast_in.tile([P, D], FP32, tag="cast", name="cfx")
                    nc.sync.dma_start(out=cfx[:], in_=x_ext[b][ts(c, P), :])
                    cbx = cast_out.tile([P, D], BF16, tag="castb", name="cbx")
                    if c % 2 == 0:
                        nc.scalar.copy(cbx[:], cfx[:])
                    else:
                        nc.vector.tensor_copy(cbx[:], cfx[:])
                    if b == 0:
                        for dt in range(DT):
                            pt = psum_bt.tile([P, P], BF16, tag="bt",
                                              name="ptx")
                            nc.tensor.matmul(pt[:], cbx[:, ts(dt, P)],
                                             id128[:], is_transpose=True,
                                             start=True, stop=True)
                            dst = xT[0][:, dt, ts(c, P)]
                            if dt % 2 == 0:
                                nc.scalar.copy(dst, pt[:])
                            else:
                                nc.vector.tensor_copy(dst, pt[:])
                    else:
                        nc.sync.dma_start(out=xbf1[ts(c, P), :], in_=cbx[:])
                # y matmuls: b0 reads x2T slices; b1 streamed x2s chunks
                if b == 1:
                    x2s = x2s_p.tile([P, DT, QC], BF16, name="x2s")
                    for dt in range(DT):
                        nc.sync.dma_start_transpose(
                            x2s[:, dt, :], x2bf[1][ts(kc, QC), ts(dt, P)])
                pw = psum_cw.tile([1, QC], FP32, tag="cw", name="pw")
                for ep in range(DT // 2):
                    yt = yt_p.tile([P, 2, QC], BF16, name="yt")
                    for e2 in range(2):
                        et = 2 * ep + e2
                        ps = psum_a.tile([P, QC], FP32, tag="psa",
                                         name="ps_y")
                        for dt in range(DT):
                            rhs = (x2T[0][:, dt, ts(kc, QC)] if b == 0
                                   else x2s[:, dt, :])
                            nc.tensor.matmul(
                                ps[:], W_sb[:, dt, ts(et, P)], rhs,
                                start=(dt == 0), stop=(dt == DT - 1))
                        nc.scalar.activation(yt[:, e2, :], ps[:], AF.Tanh)
                    for e2 in range(2):
                        et = 2 * ep + e2
                        nc.tensor.matmul(
                            pw[:], v_sb[:, et:et + 1], yt[:, e2, :],
                            start=(et == 0), stop=(et == DT - 1))
                nc.scalar.activation(
                    exp_wlog[b][:, ts(kc, QC)], pw[:], AF.Exp)

        # ---- AllReduce of sum_b exp(wlog) ----
        partial = rows_t.tile([1, L], FP32, tag="row", name="partial")
        nc.vector.tensor_add(partial[:], exp_wlog[0][:], exp_wlog[1][:])
        nc.sync.dma_start(out=ar_in[:], in_=partial[:])
        nc.gpsimd.collective_compute(
            "AllReduce", mybir.AluOpType.add,
            replica_groups=[list(range(NCORES))],
            ins=[ar_in[:].opt()], outs=[ar_out.ap().opt()])

        # denom on the scalar queue (its wait must not block other DMAs);
        # reciprocal in [128, 16] layout (single-lane recip is ~15us),
        # scattered back to a row via DRAM
        denom_pm = m_p.tile([P, 16], FP32, tag="dpm", name="denom_pm")
        nc.scalar.dma_start(
            out=denom_pm[:],
            in_=ar_out.ap()[0, :].rearrange("(p i) -> p i", i=16))
        recip_pm = m_p.tile([P, 16], FP32, tag="rpm", name="recip_pm")
        nc.vector.reciprocal(recip_pm[:], denom_pm[:])
        nc.scalar.dma_start(
            out=recd[0, :].rearrange("(p i) -> p i", i=16), in_=recip_pm[:])
        nc.scalar.dma_start(out=recip[:], in_=recd[:])

        # ---- per batch: weights, outT (in place on x2T), out, attention ----
        for b in range(NB):
            w_row = rows_t.tile([1, L], FP32, tag="row", name=f"w_row{b}")
            nc.vector.tensor_mul(w_row[:], exp_wlog[b][:], recip[:])

            # M[p, jt, r] = w_row[(128jt + 768r) % 2048 + p] via 48 K=1
            # rank-1 matmuls (offsets are 128-aligned and never wrap)
            M_sb = m_p.tile([P, DT, 8], FP32, tag="m", name="M_sb")
            for jt in range(DT):
                pm = psum_cw.tile([P, 8], FP32, tag="cw", name="pm")
                for r in range(8):
                    o = (128 * jt + 768 * r) % L
                    nc.tensor.matmul(pm[:, r:r + 1], w_row[:, o:o + P],
                                     one1[:], start=True, stop=True)
                nc.vector.tensor_copy(M_sb[:, jt, :], pm[:])
            # outT in place: x2T[:, jt, r::8] *= M[:, jt, r]
            # split across DVE (tensor_scalar) and ACT (Copy with scale=)
            for jt in range(DT):
                sl = x2T[b][:, jt, :].rearrange("p (k e) -> p k e", e=8)
                for r in range(8):
                    if r % 2 == 0:
                        nc.vector.tensor_scalar_mul(
                            sl[:, :, r], sl[:, :, r], M_sb[:, jt, r:r + 1])
                    else:
                        nc.scalar.activation(
                            sl[:, :, r], sl[:, :, r], AF.Copy,
                            scale=M_sb[:, jt, r:r + 1])
            outT = x2T[b]

            # broadcast w to 128 partitions (bf16) for the flat multiply
            wb = wb_p.tile([P, L], BF16, name="wb")
            for c in range(NQC):
                psb = psum_a.tile([P, QC], FP32, tag="psa", name="psb")
                nc.tensor.matmul(psb[:], ones_sb[:], w_row[:, ts(c, QC)],
                                 start=True, stop=True)
                nc.vector.tensor_copy(wb[:, ts(c, QC)], psb[:])
            x2fl = x2bf[b][:].rearrange("l d -> (l d)").rearrange(
                "(r c) -> r c", c=L)
            ofl = outbf[b][:].rearrange("l d -> (l d)").rearrange(
                "(r c) -> r c", c=L)
            for j in range(DT):
                for h in range(2):
                    hs = ts(h, L // 2)
                    xf = flat_p.tile([P, L // 2], BF16, name="xf")
                    nc.sync.dma_start(out=xf[:], in_=x2fl[ts(j, P), hs])
                    of = oflat_p.tile([P, L // 2], BF16, name="of")
                    nc.vector.tensor_mul(of[:], xf[:], wb[:, hs])
                    nc.sync.dma_start(out=ofl[ts(j, P), hs], in_=of[:])

            # V with ones column: oa[128, kt, 769]
            oa = oa_p.tile([P, KT, D + 1], BF16, tag="oa", name="oa")
            for g in range(4):   # 4 k-tiles per DMA
                src = outbf[b][ts(g, 4 * P), :].rearrange(
                    "(t p) d -> p t d", p=P)
                nc.sync.dma_start(out=oa[:, 4 * g:4 * g + 4, 0:D], in_=src)
            nc.vector.memset(oa[:, :, D:D + 1], 1.0)

            for qc in range(NQC):
                # QK into per-kt expT ring tiles
                expT = []
                for kt in range(KT):
                    ps = psum_a.tile([P, QC], FP32, tag="psa", name="ps_qk")
                    for dt in range(DT):
                        nc.tensor.matmul(
                            ps[:], outT[:, dt, ts(kt, P)],
                            xT[b][:, dt, ts(qc, QC)],
                            start=(dt == 0), stop=(dt == DT - 1))
                    et = expT_p.tile([P, QC], BF16, tag="expT", name="et")
                    nc.scalar.activation(et[:], ps[:], AF.Exp)
                    expT.append(et)
                # PV: kt-major over pairs of q-tiles (4 psum tiles live)
                for qh in range(QC // P // 2):
                    pcs = []
                    for qt in (2 * qh, 2 * qh + 1):
                        pc1 = psum_bt.tile([P, 512], FP32, tag="bt",
                                           name="pc1")
                        pc2 = psum_cw.tile([P, 257], FP32, tag="cw",
                                           name="pc2")
                        pcs.append((qt, pc1, pc2))
                    for kt in range(KT):
                        for qt, pc1, pc2 in pcs:
                            lh = expT[kt][:, ts(qt, P)]
                            nc.tensor.matmul(
                                pc1[:], lh, oa[:, kt, 0:512],
                                start=(kt == 0), stop=(kt == KT - 1))
                            nc.tensor.matmul(
                                pc2[:], lh, oa[:, kt, 512:D + 1],
                                start=(kt == 0), stop=(kt == KT - 1))
                    for qt, pc1, pc2 in pcs:
                        rec = rec_p.tile([P, 1], FP32, name="rec")
                        nc.vector.reciprocal(rec[:], pc2[:, 256:257])
                        cc = ctx_p.tile([P, D], FP32, tag="cc", name="cc")
                        nc.vector.tensor_scalar_mul(
                            cc[:, 0:512], pc1[:], rec[:])
                        nc.vector.tensor_scalar_mul(
                            cc[:, 512:D], pc2[:, 0:256], rec[:])
                        q0 = qc * QC + qt * P
                        nc.sync.dma_start(
                            out=out_ext[b, q0:q0 + P, :], in_=cc[:])
                # prefetch batch-1 xT via DMA transposes at the tail of
                # batch-0's attention (sequencer is idle by then)
                if b == 0 and qc == NQC - 1:
                    xT[1] = xt_p.tile([P, DT, L], BF16, name="xT1")
                    x2T[1] = x2t_p.tile([P, DT, L], BF16, name="x2T1")
                    for dt in range(DT):
                        nc.sync.dma_start_transpose(
                            xT[1][:, dt, :], xbf1[:, ts(dt, P)])
                    for dt in range(DT):
                        nc.sync.dma_start_transpose(
                            x2T[1][:, dt, :], x2bf[1][:, ts(dt, P)])


_NC_CACHE = None


def kernel(x, x2, att_v, att_W):
    global _NC_CACHE
    if _NC_CACHE is None:
        _NC_CACHE = build_nc()
    nc = _NC_CACHE

    x = np.ascontiguousarray(x, dtype=np.float32)
    x2 = np.ascontiguousarray(x2, dtype=np.float32)
    att_v = np.ascontiguousarray(att_v, dtype=np.float32)
    att_W = np.ascontiguousarray(att_W, dtype=np.float32)

    in_maps = []
    for i in range(NCORES):
        sl = slice(i * NB, (i + 1) * NB)
        in_maps.append({
            "x": x[sl], "x2": x2[sl], "att_v": att_v, "att_W": att_W,
        })
    res = run_bass_kernel_spmd(nc, in_maps, core_ids=list(range(NCORES)))
    outs = [res.results[i]["out"] for i in range(NCORES)]
    return np.concatenate(outs, axis=0).astype(np.float32)


if __name__ == "__main__":
    xs = np.random.randn(B, L, D).astype(np.float32)
    x2s = np.random.randn(B, L, D).astype(np.float32)
    vs = (np.random.randn(D) * 0.01).astype(np.float32)
    Ws = (np.random.randn(D, D) * 0.01).astype(np.float32)
    o = kernel(x=xs, x2=x2s, att_v=vs, att_W=Ws)
    print(o.shape, o.dtype)
